# revision 1
# baseline (speedup 1.0000x reference)
"""NCD-via-LZW kernel for Trainium2 (8 NeuronCores, Bass).

Problem: quantize x [32,3,32,32] to 8 levels along a space-filling curve =>
96 strings of length 1024; LZW-compress the 96 strings, the 48 pattern maps,
and the 1536 string||pmap concatenations; return the normalized compression
distance matrix [32, 48].

Mapping: LZW is sequential per sequence but there are 1680 independent
sequences. Each NeuronCore handles batches 4n..4n+3 (192 concat runs) plus 6
of the 48 pmap runs, one LZW sequence per SBUF partition, 4 stock DVE
instructions per sequential step.

Cross-wave prefix sharing: a lane keeps the same (b, c) in both waves
(8 of the 16 pmaps in each wave). Wave 0 runs the full 2047-step concat
string||pmap_{k0}. Wave 1 reuses the trie state wave 0 left in SBUF — entry
slots [0, 1024) and the node counter EN[1023] are exactly the state after
the shared 1024-symbol string prefix — and runs only the 1024 suffix steps
for pmap_{k1}, overwriting slots [1024, 2048) just ahead of its own match
stream. The string compressed sizes c_s are wave 0's EN[1023] column for
free; pmap-only runs use spare lanes 96..101 of wave 0.

Per-lane LZW state (all exact in fp32):
  key(cur, c) = cur + (c+1)/16  (cur = integer trie node id, c in [0,8))
  EK[t] = key inserted at step t on miss, 0.0 on hit (queries >= 1/16 > 0)
  EN[t] = 7 + (#misses after step t)  (= node id created at the last miss)
Step t (query in keyp1, match result in acc):
  1. acc = sum_j (EK[0:t] == keyp1) * EN[0:t]   -- scalar_tensor_tensor with
     fused accum; = matched node id or 0 (at most one slot matches)
  2. EN[t] = (acc == 0) + EN[t-1]
  3. EK[t] = (acc == 0) * keyp1
  4. keyp1 = max(acc, c_t) + (c_{t+1}+1)/16     (node ids >= 8 > c_t)
lzw_count(seq[0:L]) = EN[L-1] - 6.
"""

import numpy as np

B, C, H, W = 32, 3, 32, 32
L = 8
P = 16
M = 1024
N = H * W
T = 2048
PRE = 1024  # shared prefix length (the string part of each concat)
NCORES = 8

_nc_cache = {}


class _Chain:
    """Same-engine serialization via an attached-wait semaphore chain (the
    pattern Tile emits for same-engine RAW deps; required for correctness on
    this hardware — verified empirically)."""

    def __init__(self, sem):
        self.sem = sem
        self.k = 0

    def add(self, inst):
        if self.sem is not None:
            inst._wait_ge(self.sem, self.k)
            inst.then_inc(self.sem)
        self.k += 1
        return inst


def _emit_steps(vector, ch, AO, EK, EN, scratch, acc, keyp1, t0, t1, scol):
    """Emit LZW steps t = t0..t1-1. scol(t) returns the AP pair
    (c_t column, (c_{t+1}+1)/16 column)."""
    for t in range(t0, t1):
        c_col, cn_col = scol(t)
        if t >= 2:
            ch.add(vector.scalar_tensor_tensor(
                scratch[:, 0:t], EK[:, 0:t], keyp1[:], EN[:, 0:t],
                AO.is_equal, AO.mult, accum_out=acc[:]))
        ch.add(vector.scalar_tensor_tensor(
            EN[:, t:t + 1], acc[:], 0.0, EN[:, t - 1:t],
            AO.is_equal, AO.add))
        ch.add(vector.scalar_tensor_tensor(
            EK[:, t:t + 1], acc[:], 0.0, keyp1[:], AO.is_equal, AO.mult))
        ch.add(vector.scalar_tensor_tensor(
            keyp1[:], acc[:], c_col, cn_col, AO.max, AO.add))


def _build_program(use_chain=True):
    import concourse.bass as bass
    import concourse.mybir as mybir

    key = ("nc", use_chain, "shared")
    if key in _nc_cache:
        return _nc_cache[key]

    dt = mybir.dt.float32
    AO = mybir.AluOpType
    nc = bass.Bass()

    # Wave 0: full concats [128, 2T]; wave 1: suffix-only [128, 2*(T-PRE)+2]
    W1C = 2 * (T - PRE) + 2
    syms_d = [nc.declare_dram_parameter("syms0", [128, 2 * T], dt,
                                        isOutput=False),
              nc.declare_dram_parameter("syms1", [128, W1C], dt,
                                        isOutput=False)]
    out_d = nc.declare_dram_parameter("counts", [128, 3], dt, isOutput=True)

    sym0 = nc.alloc_sbuf_tensor("sym0", [128, 2 * T], dt).ap()
    sym1 = nc.alloc_sbuf_tensor("sym1", [128, W1C], dt).ap()
    EK = nc.alloc_sbuf_tensor("EK", [128, T], dt).ap()
    EN = nc.alloc_sbuf_tensor("EN", [128, T], dt).ap()
    scratch = nc.alloc_sbuf_tensor("scratch", [128, T], dt).ap()
    acc = nc.alloc_sbuf_tensor("acc", [128, 1], dt).ap()
    keyp1 = nc.alloc_sbuf_tensor("keyp1", [128, 1], dt).ap()
    curt = nc.alloc_sbuf_tensor("curt", [128, 1], dt).ap()
    outt = nc.alloc_sbuf_tensor("outt", [128, 3], dt).ap()

    dma_sem = nc.alloc_semaphore("dma_sem")
    chain_sem = nc.alloc_semaphore("chain_sem")
    done_sem = nc.alloc_semaphore("done_sem")

    # wave0: 4 init + 3 (t=1) + 4*(T-2) + 1 curt + 2 copies
    # wave1: 1 seed + 4*(T-PRE) + 1 copy
    total_chain = (4 + 3 + 4 * (T - 2) + 1 + 2) + (1 + 4 * (T - PRE) + 1)

    with nc.Block() as block:

        @block.sync
        def _(sync):
            for w in range(2):
                sync.dma_start([sym0, sym1][w][:],
                               syms_d[w][:]).then_inc(dma_sem, 16)
            if use_chain:
                sync.wait_ge(chain_sem, total_chain)
            else:
                sync.wait_ge(done_sem, 1)
            sync.dma_start(out_d[:], outt[:]).then_inc(dma_sem, 16)

        @block.vector
        def _(vector):
            vector.wait_ge(dma_sem, 32)
            ch = _Chain(chain_sem if use_chain else None)

            # ---- wave 0: full concat runs (string || pmap_{k0}) ----
            ch.add(vector.memset(acc[:], 0.0))
            ch.add(vector.memset(EK[:, 0:1], 0.0))
            ch.add(vector.memset(EN[:, 0:1], 7.0))
            ch.add(vector.scalar_tensor_tensor(
                keyp1[:], acc[:], sym0[:, 0:1], sym0[:, 1:2],
                AO.max, AO.add))

            def scol0(t):
                return (sym0[:, 2 * t:2 * t + 1],
                        sym0[:, 2 * t + 1:2 * t + 2])

            _emit_steps(vector, ch, AO, EK, EN, scratch, acc, keyp1,
                        1, PRE, scol0)
            # save cur_{PRE-1} = max(acc, c_{PRE-1}) for the wave-1 restart
            ch.add(vector.tensor_scalar(
                curt[:], acc[:], sym0[:, 2 * (PRE - 1):2 * (PRE - 1) + 1],
                None, AO.max))
            _emit_steps(vector, ch, AO, EK, EN, scratch, acc, keyp1,
                        PRE, T, scol0)
            ch.add(vector.tensor_copy(outt[:, 0:1], EN[:, PRE - 1:PRE]))
            ch.add(vector.tensor_copy(outt[:, 1:2], EN[:, T - 1:T]))

            # ---- wave 1: suffix-only runs (pmap_{k1}) reusing the prefix
            # trie state in EK/EN[0:PRE] ----
            # seed query: cur_{PRE-1} + (p'_0+1)/16
            ch.add(vector.scalar_tensor_tensor(
                keyp1[:], curt[:], 1.0, sym1[:, 0:1], AO.mult, AO.add))

            def scol1(t):
                u = t - PRE
                return (sym1[:, 2 * u + 1:2 * u + 2],
                        sym1[:, 2 * u + 2:2 * u + 3])

            _emit_steps(vector, ch, AO, EK, EN, scratch, acc, keyp1,
                        PRE, T, scol1)
            last = ch.add(vector.tensor_copy(outt[:, 2:3], EN[:, T - 1:T]))
            assert ch.k == total_chain, (ch.k, total_chain)
            if not use_chain:
                last.then_inc(done_sem)

    _nc_cache[key] = nc
    return nc


def _prestage_full(syms):
    """[n, T] symbols -> [n, 2T]: col 2t = c_t, col 2t+1 = (c_{t+1}+1)/16."""
    syms = np.asarray(syms, np.float32)
    n, T_ = syms.shape
    out = np.zeros((n, 2 * T_), np.float32)
    out[:, ::2] = syms
    out[:, 1:2 * T_ - 2:2] = (syms[:, 1:] + 1.0) / 16.0
    return out


def _prestage_suffix(syms):
    """[n, S] suffix symbols -> [n, 2S+2]: col 0 = (c_0+1)/16,
    col 2u+1 = c_u, col 2u+2 = (c_{u+1}+1)/16 (0 for the last)."""
    syms = np.asarray(syms, np.float32)
    n, S = syms.shape
    out = np.zeros((n, 2 * S + 2), np.float32)
    out[:, 0] = (syms[:, 0] + 1.0) / 16.0
    out[:, 1::2][:, :S] = syms
    out[:, 2:2 * S - 1:2] = (syms[:, 1:] + 1.0) / 16.0
    return out


def _quantize(x, curve, levels):
    """x [B,C,H,W] -> strings [B,C,N] int32 (nearest level, first-min)."""
    out = np.asarray(x, np.float32).reshape(B, C, -1)[:, :, np.asarray(curve)]
    lv = np.asarray(levels, np.float32)
    return np.argmin(
        np.abs(out[:, :, None, :] - lv[:, None].reshape(1, C, L, 1)), axis=2
    ).astype(np.int32)


def _lane_symbols(strings, pmaps):
    """Per-core symbol matrices.

    Core n, lanes 0..95: bc = lane//8 (b = 4n + bc//3, c = bc%3),
    k0 = lane%8 (wave 0 concat), k1 = 8 + lane%8 (wave 1 suffix).
    Lanes 96..101 (wave 0): pmap-only runs cp = 6n..6n+5, zero-padded.
    Returns (syms0, syms1) lists of [128, *] f32 arrays."""
    pm = np.asarray(pmaps, np.int64)
    syms0, syms1 = [], []
    for n in range(NCORES):
        w0 = np.zeros((128, T), np.int64)
        w1 = np.zeros((128, T - PRE), np.int64)
        for lane in range(96):
            bc, k = lane // 8, lane % 8
            b_loc, c = bc // 3, bc % 3
            s = strings[4 * n + b_loc, c]
            w0[lane] = np.concatenate([s, pm[c, k]])
            w1[lane] = pm[c, 8 + k]
        for jj in range(6):
            cp = 6 * n + jj
            w0[96 + jj, :M] = pm[cp // 16, cp % 16]
        syms0.append(_prestage_full(w0))
        syms1.append(_prestage_suffix(w1))
    return syms0, syms1


def _assemble(results):
    """results[n]['counts'] [128, 3] -> ncd [32, 48] f32.
    cols: 0 = EN[PRE-1] (c_s / c_p), 1 = wave0 EN[T-1] (c_sp k0),
    2 = wave1 EN[T-1] (c_sp k1)."""
    c_s = np.zeros((B, C), np.float32)
    c_p = np.zeros((C, P), np.float32)
    c_sp = np.zeros((B, C, P), np.float32)
    for n in range(NCORES):
        cnts = np.asarray(results[n]["counts"], np.float32) - 6.0
        for lane in range(96):
            bc, k = lane // 8, lane % 8
            b_loc, c = bc // 3, bc % 3
            c_sp[4 * n + b_loc, c, k] = cnts[lane, 1]
            c_sp[4 * n + b_loc, c, 8 + k] = cnts[lane, 2]
            if k == 0:
                c_s[4 * n + b_loc, c] = cnts[lane, 0]
        for jj in range(6):
            cp = 6 * n + jj
            c_p[cp // 16, cp % 16] = cnts[96 + jj, 0]
    ncd = (c_sp - np.minimum(c_s[:, :, None], c_p[None, :, :])) / np.maximum(
        c_s[:, :, None], c_p[None, :, :])
    return ncd.reshape(B, C * P).astype(np.float32)


def _run(in_maps, trace=False):
    from concourse.bass_utils import run_bass_kernel_spmd
    nc = _build_program()
    return run_bass_kernel_spmd(nc, in_maps, list(range(NCORES)), trace=trace)


def _in_maps(x, curve, levels, pmaps):
    strings = _quantize(x, curve, levels)
    syms0, syms1 = _lane_symbols(strings, pmaps)
    return [{"syms0": syms0[n], "syms1": syms1[n]} for n in range(NCORES)]


def kernel(x, curve, levels, pmaps, i=0, **_unused):
    del i
    in_maps = _in_maps(x, curve, levels, pmaps)
    res = _run(in_maps)
    return _assemble([res.results[n] for n in range(NCORES)])


def kernel_profiled(x, curve, levels, pmaps, i=0, **_unused):
    """Like kernel() but with NTFF tracing; returns (out, exec_time_ns).
    Falls back to (out, None) when the profiling hook is unavailable."""
    del i
    in_maps = _in_maps(x, curve, levels, pmaps)
    try:
        res = _run(in_maps, trace=True)
        return (_assemble([res.results[n] for n in range(NCORES)]),
                res.exec_time_ns)
    except Exception:
        res = _run(in_maps)
        return _assemble([res.results[n] for n in range(NCORES)]), None

